# revision 21
# baseline (speedup 1.0000x reference)
"""DepthConv kernel for Trainium2 (Bass/Tile), data-parallel over batch on 8 cores.

Problem: out[b,o,x,y] = sum_{c,k} w[o,c,k] * data[b,c,x+i,y+j] * aff[b,k,x,y]
         aff[b,k,x,y] = exp(-8.3*|depth[b,x+i,y+j] - depth[b,x+1,y+1]|), k=(i,j) in 3x3
Shapes: data [8,16,256,256], depth [8,1,256,256], weight [16,16,3,3] -> out [8,16,254,254]

Per-core layout (1 image/core): partitions = (strip q=0..7, channel c=0..15).
Each strip covers 32 output rows; free dim n = xl*256+y (flat, row-wrapping).
 - 3x3 taps become pure free-dim shifts (i*256+j) of one resident data tile.
 - Per-tap matmul uses block-diagonal weights [(q,c),(q,o)] so all 8 strips'
   channel contractions run in one full-width 128x128 matmul; 9 taps
   PSUM-accumulate.
 - Affinity aff[(q,k),n] is computed per n-tile (PE center-selection matmul +
   DVE sub + ACT abs/exp), then replicated across the 16 channel rows of each
   strip via a selection-matrix matmul on the PE (output straight into PSUM,
   consumed by the DVE multiply).
 - float32r matmuls (full PE rate at N=512, fp32 storage).
 - The entire input (data windows, pre-shifted depth taps, weight/selection
   matrices) is packed host-side into ONE [128, TOT] tensor loaded by ONE DMA,
   and each tile stores with ONE DMA — keeps every instruction's semaphore
   wait count within walrus's tiny per-instruction limits.
"""

import numpy as np

B, C, H, W = 8, 16, 256, 256
O, KH, KW = 16, 3, 3
ALPHA = 8.3
OH, OW = H - KH + 1, W - KW + 1  # 254, 254
P = 128
NQ, QROWS = 8, 32           # strips, output rows per strip
NFREE = QROWS * W           # 8192 flat pixels per strip (incl. y>=254 garbage)
NTILE = 512
NT = NFREE // NTILE         # 16 n-tiles (2 output rows each)
DWIN = 34 * W + 16          # data window: 34 rows halo + shift pad
TAPS = [(i, j) for i in range(KH) for j in range(KW)]
NC_KS = [k for k in range(9) if k != 4]  # non-center taps
NBLK = 18                   # 9 weight blocks + 8 tap-select + 1 center-select
D0 = 0                      # data window offset in the packed tensor
Z0 = DWIN                   # dep_t offset
M0 = DWIN + NFREE           # wsmat offset
TOT = DWIN + NFREE + NBLK * P

_CACHE = {}


def _build_nc():
    import concourse.bass as bass
    import concourse.bacc as bacc
    import concourse.mybir as mybir
    from concourse.tile import TileContext
    from concourse.alu_op_type import AluOpType
    from concourse.bass_types import AP

    f32 = mybir.dt.float32
    f32r = mybir.dt.float32r
    AF = mybir.ActivationFunctionType

    nc = bacc.Bacc(None, target_bir_lowering=False)
    allin_d = nc.dram_tensor("allin", [P, TOT], f32r, kind="ExternalInput")
    out_d = nc.dram_tensor("out", [O, OH, OW], f32, kind="ExternalOutput")
    out_flat = out_d[:].flatten()

    with TileContext(nc) as tc:
        with (
            tc.tile_pool(name="const", bufs=1) as cpool,
            tc.tile_pool(name="vpool", bufs=6) as vpool,
            tc.tile_pool(name="opool", bufs=4) as opool,
            tc.tile_pool(name="zpool", bufs=3) as zpool,
            tc.tile_pool(name="affps", bufs=4, space="PSUM") as affps,
            tc.tile_pool(name="outps", bufs=3, space="PSUM") as outps,
        ):
            allin = cpool.tile([P, TOT], f32r)
            osb_all = cpool.tile([P, NFREE], f32)
            nc.sync.dma_start(allin[:], allin_d[:])

            def seg(off, size):
                return allin[:, off : off + size]

            def mk(base_ap, extra_off, dims):
                return AP(base_ap.tensor, base_ap.offset + extra_off, dims)

            for t in range(NT):
                base = t * NTILE
                # center-depth replicated over the 16 rows of each strip
                zc_ps = affps.tile([P, NTILE], f32, tag="affps")
                nc.tensor.matmul(
                    zc_ps[:],
                    seg(M0 + 17 * P, P),
                    seg(Z0 + base, NTILE),
                    start=True,
                    stop=True,
                )
                afft = zpool.tile([P, NTILE], f32r, tag="afft")
                nc.scalar.activation(afft[:], zc_ps[:], AF.Abs, scale=-ALPHA)
                nc.scalar.activation(afft[:], afft[:], AF.Exp, scale=-1.0)

                outp = outps.tile([P, NTILE], f32, tag="outp")
                for idx, k in enumerate(range(9)):
                    i, j = TAPS[k]
                    shift = base + i * W + j
                    if k == 4:
                        rhs = seg(D0 + shift, NTILE)
                    else:
                        jj = NC_KS.index(k)
                        ap_ps = affps.tile([P, NTILE], f32, tag="affps")
                        nc.tensor.matmul(
                            ap_ps[:],
                            seg(M0 + (9 + jj) * P, P),
                            afft[:],
                            start=True,
                            stop=True,
                        )
                        v = vpool.tile([P, NTILE], f32r, tag="v")
                        nc.vector.tensor_tensor(
                            v[:], seg(D0 + shift, NTILE), ap_ps[:], AluOpType.mult
                        )
                        rhs = v[:]
                    nc.tensor.matmul(
                        outp[:],
                        seg(M0 + k * P, P),
                        rhs,
                        start=(idx == 0),
                        stop=(idx == 8),
                        skip_group_check=True,
                    )
                # evacuate PSUM into the full-image staging buffer
                nc.scalar.copy(
                    osb_all[:, base : base + NTILE], outp[:]
                )
            for q in range(NQ):
                nrows = min(QROWS, OH - 32 * q)
                src_ap = osb_all[16 * q : 16 * q + 16, :].rearrange(
                    "o (x y) -> o x y", y=W
                )[:, 0:nrows, 0:OW]
                nc.sync.dma_start(
                    out_d[:, 32 * q : 32 * q + nrows, :], src_ap
                )
    nc.compile()
    return nc


def _pack_inputs(data, depth, weight):
    """Build the [B, 128, TOT] packed input: data windows, shifted depth
    taps, and the weight/selection matrices."""
    HP = H + 3
    data_p = np.zeros((B, C, HP * W), np.float32)
    data_p[:, :, : H * W] = data.reshape(B, C, H * W)
    depth_p = np.zeros((B, HP * W), np.float32)
    depth_p[:, : H * W] = depth.reshape(B, H * W)

    wsmat = np.zeros((NBLK, P, P), np.float32)
    for k in range(9):
        i, j = TAPS[k]
        blk = weight[:, :, i, j].T  # [c, o]
        for q in range(NQ):
            wsmat[k, 16 * q : 16 * q + 16, 16 * q : 16 * q + 16] = blk
    for jj, k in enumerate(NC_KS):
        for q in range(NQ):
            wsmat[9 + jj, 16 * q + k, 16 * q : 16 * q + 16] = 1.0
    wsmat[17] = np.eye(P, dtype=np.float32)
    for q in range(NQ):
        wsmat[17, 16 * q + 4, 16 * q : 16 * q + 16] -= 1.0
    wsmat_flat = wsmat.transpose(1, 0, 2).reshape(P, NBLK * P)

    allin = np.zeros((B, P, TOT), np.float32)
    for q in range(NQ):
        for c in range(C):
            p = 16 * q + c
            s = 32 * q * W
            allin[:, p, D0 : D0 + DWIN] = data_p[:, c, s : s + DWIN]
        for k, (i, j) in enumerate(TAPS):
            p = 16 * q + k
            s = (32 * q + i) * W + j
            allin[:, p, Z0 : Z0 + NFREE] = depth_p[:, s : s + NFREE]
    allin[:, :, M0:] = wsmat_flat[None]
    return allin


def run(inputs, **spmd_kwargs):
    from concourse.bass_utils import run_bass_kernel_spmd

    data = np.asarray(inputs["data"], np.float32)
    depth = np.asarray(inputs["depth"], np.float32)
    weight = np.asarray(inputs["weight"], np.float32)
    allin = _pack_inputs(data, depth, weight)

    if "nc" not in _CACHE:
        _CACHE["nc"] = _build_nc()
    nc = _CACHE["nc"]

    in_maps = [{"allin": np.ascontiguousarray(allin[b])} for b in range(B)]
    res = run_bass_kernel_spmd(nc, in_maps, core_ids=list(range(B)), **spmd_kwargs)
    out = np.stack([res.results[b]["out"] for b in range(B)]).astype(np.float32)
    return out, res


def kernel(**inputs):
    out, _ = run(inputs)
    return out


# revision 22
# speedup vs baseline: 1.0389x; 1.0389x over previous
"""DepthConv kernel for Trainium2 (Bass/Tile), data-parallel over batch on 8 cores.

Problem: out[b,o,x,y] = sum_{c,k} w[o,c,k] * data[b,c,x+i,y+j] * aff[b,k,x,y]
         aff[b,k,x,y] = exp(-8.3*|depth[b,x+i,y+j] - depth[b,x+1,y+1]|), k=(i,j) in 3x3
Shapes: data [8,16,256,256], depth [8,1,256,256], weight [16,16,3,3] -> out [8,16,254,254]

Per-core layout (1 image/core): partitions = (strip q=0..7, channel c=0..15).
Each strip covers 32 output rows; free dim n = xl*256+y (flat, row-wrapping).
 - 3x3 taps become pure free-dim shifts (i*256+j) of one resident data tile.
 - Per-tap matmul uses block-diagonal weights [(q,c),(q,o)] so all 8 strips'
   channel contractions run in one full-width 128x128 matmul; 9 taps
   PSUM-accumulate.
 - Affinity aff[(q,k),n] is computed per n-tile (PE center-selection matmul +
   DVE sub + ACT abs/exp), then replicated across the 16 channel rows of each
   strip via a selection-matrix matmul on the PE (output straight into PSUM,
   consumed by the DVE multiply).
 - float32r matmuls (full PE rate at N=512, fp32 storage).
 - The entire input (data windows, pre-shifted depth taps, weight/selection
   matrices) is packed host-side into ONE [128, TOT] tensor loaded by ONE DMA,
   and each tile stores with ONE DMA — keeps every instruction's semaphore
   wait count within walrus's tiny per-instruction limits.
"""

import numpy as np

B, C, H, W = 8, 16, 256, 256
O, KH, KW = 16, 3, 3
ALPHA = 8.3
OH, OW = H - KH + 1, W - KW + 1  # 254, 254
P = 128
NQ, QROWS = 8, 32           # strips, output rows per strip
NFREE = QROWS * W           # 8192 flat pixels per strip (incl. y>=254 garbage)
NTILE = 512
NT = NFREE // NTILE         # 16 n-tiles (2 output rows each)
DWIN = 34 * W + 16          # data window: 34 rows halo + shift pad
TAPS = [(i, j) for i in range(KH) for j in range(KW)]
NC_KS = [k for k in range(9) if k != 4]  # non-center taps
NBLK = 18                   # 9 weight blocks + 8 tap-select + 1 center-select
D0 = 0                      # data window offset in the packed tensor
Z0 = DWIN                   # dep_t offset
M0 = DWIN + NFREE           # wsmat offset
TOT = DWIN + NFREE + NBLK * P

_CACHE = {}


def _build_nc():
    import concourse.bass as bass
    import concourse.bacc as bacc
    import concourse.mybir as mybir
    from concourse.tile import TileContext
    from concourse.alu_op_type import AluOpType
    from concourse.bass_types import AP

    f32 = mybir.dt.float32
    f32r = mybir.dt.float32r
    AF = mybir.ActivationFunctionType

    nc = bacc.Bacc(None, target_bir_lowering=False)
    allin_d = nc.dram_tensor("allin", [P, TOT], f32r, kind="ExternalInput")
    out_d = nc.dram_tensor("out", [O, OH, OW], f32, kind="ExternalOutput")
    out_flat = out_d[:].flatten()

    with TileContext(nc) as tc:
        with (
            tc.tile_pool(name="const", bufs=1) as cpool,
            tc.tile_pool(name="vpool", bufs=6) as vpool,
            tc.tile_pool(name="opool", bufs=4) as opool,
            tc.tile_pool(name="zpool", bufs=3) as zpool,
            tc.tile_pool(name="affps", bufs=4, space="PSUM") as affps,
            tc.tile_pool(name="outps", bufs=3, space="PSUM") as outps,
        ):
            allin = cpool.tile([P, TOT], f32r)
            osb_all = cpool.tile([P, NFREE], f32)
            nc.sync.dma_start(allin[:], allin_d[:])

            def seg(off, size):
                return allin[:, off : off + size]

            def mk(base_ap, extra_off, dims):
                return AP(base_ap.tensor, base_ap.offset + extra_off, dims)

            for t in range(NT):
                base = t * NTILE
                # center-depth replicated over the 16 rows of each strip
                zc_ps = affps.tile([P, NTILE], f32, tag="affps")
                nc.tensor.matmul(
                    zc_ps[:],
                    seg(M0 + 17 * P, P),
                    seg(Z0 + base, NTILE),
                    start=True,
                    stop=True,
                )
                afft = zpool.tile([P, NTILE], f32r, tag="afft")
                nc.scalar.activation(afft[:], zc_ps[:], AF.Abs, scale=-ALPHA)
                nc.scalar.activation(afft[:], afft[:], AF.Exp, scale=-1.0)

                outp = outps.tile([P, NTILE], f32, tag="outp")
                for idx, k in enumerate(range(9)):
                    i, j = TAPS[k]
                    shift = base + i * W + j
                    if k == 4:
                        rhs = seg(D0 + shift, NTILE)
                    else:
                        jj = NC_KS.index(k)
                        ap_ps = affps.tile([P, NTILE], f32, tag="affps")
                        nc.tensor.matmul(
                            ap_ps[:],
                            seg(M0 + (9 + jj) * P, P),
                            afft[:],
                            start=True,
                            stop=True,
                        )
                        v = vpool.tile([P, NTILE], f32r, tag="v")
                        nc.vector.tensor_tensor(
                            v[:], seg(D0 + shift, NTILE), ap_ps[:], AluOpType.mult
                        )
                        rhs = v[:]
                    nc.tensor.matmul(
                        outp[:],
                        seg(M0 + k * P, P),
                        rhs,
                        start=(idx == 0),
                        stop=(idx == 8),
                        skip_group_check=True,
                    )
                # evacuate PSUM into the full-image staging buffer
                nc.scalar.copy(
                    osb_all[:, base : base + NTILE], outp[:]
                )
                if t % 4 == 3:
                    x0 = 2 * (t - 3)
                    for q in range(NQ):
                        nrows = max(0, min(x0 + 8, OH - 32 * q) - x0)
                        if nrows == 0:
                            continue
                        src_ap = osb_all[16 * q : 16 * q + 16, :].rearrange(
                            "o (x y) -> o x y", y=W
                        )[:, x0 : x0 + nrows, 0:OW]
                        nc.sync.dma_start(
                            out_d[:, 32 * q + x0 : 32 * q + x0 + nrows, :], src_ap
                        )
    nc.compile()
    return nc


def _pack_inputs(data, depth, weight):
    """Build the [B, 128, TOT] packed input: data windows, shifted depth
    taps, and the weight/selection matrices."""
    HP = H + 3
    data_p = np.zeros((B, C, HP * W), np.float32)
    data_p[:, :, : H * W] = data.reshape(B, C, H * W)
    depth_p = np.zeros((B, HP * W), np.float32)
    depth_p[:, : H * W] = depth.reshape(B, H * W)

    wsmat = np.zeros((NBLK, P, P), np.float32)
    for k in range(9):
        i, j = TAPS[k]
        blk = weight[:, :, i, j].T  # [c, o]
        for q in range(NQ):
            wsmat[k, 16 * q : 16 * q + 16, 16 * q : 16 * q + 16] = blk
    for jj, k in enumerate(NC_KS):
        for q in range(NQ):
            wsmat[9 + jj, 16 * q + k, 16 * q : 16 * q + 16] = 1.0
    wsmat[17] = np.eye(P, dtype=np.float32)
    for q in range(NQ):
        wsmat[17, 16 * q + 4, 16 * q : 16 * q + 16] -= 1.0
    wsmat_flat = wsmat.transpose(1, 0, 2).reshape(P, NBLK * P)

    allin = np.zeros((B, P, TOT), np.float32)
    for q in range(NQ):
        for c in range(C):
            p = 16 * q + c
            s = 32 * q * W
            allin[:, p, D0 : D0 + DWIN] = data_p[:, c, s : s + DWIN]
        for k, (i, j) in enumerate(TAPS):
            p = 16 * q + k
            s = (32 * q + i) * W + j
            allin[:, p, Z0 : Z0 + NFREE] = depth_p[:, s : s + NFREE]
    allin[:, :, M0:] = wsmat_flat[None]
    return allin


def run(inputs, **spmd_kwargs):
    from concourse.bass_utils import run_bass_kernel_spmd

    data = np.asarray(inputs["data"], np.float32)
    depth = np.asarray(inputs["depth"], np.float32)
    weight = np.asarray(inputs["weight"], np.float32)
    allin = _pack_inputs(data, depth, weight)

    if "nc" not in _CACHE:
        _CACHE["nc"] = _build_nc()
    nc = _CACHE["nc"]

    in_maps = [{"allin": np.ascontiguousarray(allin[b])} for b in range(B)]
    res = run_bass_kernel_spmd(nc, in_maps, core_ids=list(range(B)), **spmd_kwargs)
    out = np.stack([res.results[b]["out"] for b in range(B)]).astype(np.float32)
    return out, res


def kernel(**inputs):
    out, _ = run(inputs)
    return out
